# revision 10
# baseline (speedup 1.0000x reference)
"""Trainium2 Bass kernel for nn_Classifier (25-step LIF SNN, 784->64->64->10).

Strategy (pure data parallel over batch, 8 cores, Bloc=2048 rows/core):

Host side (numpy, inside kernel()): shard batch, pre-transpose x -> xT,
build layout constants (W1^T, blockdiag(W2^T), blockdiag(W3^T), identity,
beta*I, -beta*I, packed biases).

Device side per core, "packed feature-major" layout for layers 1/2:
tensors are [128 partitions = 2 groups x 64 features, 1024 cols], where
column c of group g is local batch row g*1024 + c.

  phase 1:  x1 = W1 @ xT + b1 accumulated over 7 K-chunks on the PE,
            bias fused into the PSUM->SBUF copy on the scalar engine.
  step loop (t = 0..24), split into 2 column-chunks of 512:
    L1: m1' = beta*I@m1 + (-beta*I)@s1_prev + I@x1   (PE, PSUM accumulate)
        m1 SBUF copy (ACT), s1 = m1' > 1 (DVE)
    L2: m2' = beta*I@m2 + (-beta*I)@s2_prev + W2bd@s1 (PE), +b2 fused in
        the ACT copy, s2 = m2' > 1 (DVE)
    L3: h3 = W3bd@s2 (PE), +b3 in ACT copy; transposed to batch-major and
        the LIF recurrence runs on DVE (scalar_tensor_tensor etc).
    Outputs: m1'/m2' are transposed to batch-major via PE transpose-mode;
    spikes (DVE is_gt) and surrogate sigmoids (ACT) are computed from the
    transposed PSUM and streamed to DRAM per step via HWDGE DMA.

The membrane recurrence itself stays bit-faithful to the reference except
for matmul accumulation order (unavoidable) and the PSUM-accumulated
beta-decay (<=1-2 ulp/step).
"""

from contextlib import ExitStack

import numpy as np

import concourse.bass as bass
import concourse.bacc as bacc
import concourse.mybir as mybir
import concourse.tile as tile

F32 = mybir.dt.float32
BETA = 0.95
THR = 1.0

# full-size problem constants
B_FULL = 16384
N_CORES = 8
KIN_FULL = 784
F_HID = 64
O_OUT = 10
STEPS_FULL = 25


class Cfg:
    def __init__(self, Bloc, Kin, steps, CH):
        self.Bloc = Bloc          # local batch rows per core
        self.Kin = Kin            # input features (784)
        self.steps = steps
        self.G = 2                # packed groups on partitions
        self.F = F_HID
        self.O = O_OUT
        self.P = self.G * self.F  # 128
        self.PO = self.G * self.O  # 20
        assert Bloc % self.G == 0
        self.COLS = Bloc // self.G   # packed columns
        self.CH = CH                 # column chunk per matmul/psum tile
        assert self.COLS % CH == 0
        self.NCH = self.COLS // CH
        self.TP = 128                # transpose block width (input cols)
        assert CH % self.TP == 0
        self.TC = CH // self.TP      # transpose blocks per chunk
        # K chunking for phase 1
        self.kcs = []
        k = Kin
        while k > 0:
            c = min(128, k)
            self.kcs.append(c)
            k -= c


def build_nc(cfg: Cfg):
    """Builds the per-core SPMD Bass program. Returns (nc, in_names, out_names)."""
    G, F, O, P, PO = cfg.G, cfg.F, cfg.O, cfg.P, cfg.PO
    COLS, CH, NCH, TP, TC = cfg.COLS, cfg.CH, cfg.NCH, cfg.TP, cfg.TC
    steps, Kin, Bloc = cfg.steps, cfg.Kin, cfg.Bloc
    GF = G * F
    GO = G * O
    AL = mybir.AluOpType

    nc = bacc.Bacc("TRN2", target_bir_lowering=False, debug=False,
                   enable_asserts=False)

    # ---- DRAM parameters (per core) ----
    xT_d = nc.declare_dram_parameter("xT", [Kin, Bloc], F32, isOutput=False)
    w1t_d = nc.declare_dram_parameter("w1t", [Kin, F], F32, isOutput=False)
    w2bd_d = nc.declare_dram_parameter("w2bd", [P, P], F32, isOutput=False)
    w3bd_d = nc.declare_dram_parameter("w3bd", [P, PO], F32, isOutput=False)
    eye_d = nc.declare_dram_parameter("eye", [P, P], F32, isOutput=False)
    beye_d = nc.declare_dram_parameter("beye", [P, P], F32, isOutput=False)
    nbeye_d = nc.declare_dram_parameter("nbeye", [P, P], F32, isOutput=False)
    b1_d = nc.declare_dram_parameter("b1p", [P, 1], F32, isOutput=False)
    b2_d = nc.declare_dram_parameter("b2p", [P, 1], F32, isOutput=False)
    b3_d = nc.declare_dram_parameter("b3p", [PO, 1], F32, isOutput=False)

    spko_d = nc.declare_dram_parameter("spk_out", [steps, Bloc, O], F32, isOutput=True)
    spk1_d = nc.declare_dram_parameter("spk1", [steps, Bloc, F], F32, isOutput=True)
    spk2_d = nc.declare_dram_parameter("spk2", [steps, Bloc, F], F32, isOutput=True)
    ss1_d = nc.declare_dram_parameter("ss1", [steps, Bloc, F], F32, isOutput=True)
    ss2_d = nc.declare_dram_parameter("ss2", [steps, Bloc, F], F32, isOutput=True)
    ss3_d = nc.declare_dram_parameter("ss3", [steps, Bloc, O], F32, isOutput=True)

    # batch-major DRAM views. Packed column c = j*TP + u (j = global
    # 128-col block) of partition-group g maps to local batch row
    # r = j*(G*TP) + g*TP + u, so per-(t, h) DMA APs stay 3-dimensional
    # (the k and g strides merge: g stride TP*f, k stride G*TP*f).
    def bview(d):
        return d[:].rearrange("t (h k g c) f -> t h c k g f",
                              h=NCH, k=TC, g=G, c=TP)

    v_spk1, v_spk2, v_ss1, v_ss2 = map(bview, (spk1_d, spk2_d, ss1_d, ss2_d))
    v_spko, v_ss3 = map(bview, (spko_d, ss3_d))

    with tile.TileContext(nc) as tc, ExitStack() as es:
        cpool = es.enter_context(tc.tile_pool(name="const", bufs=1))
        eye_s = cpool.tile([P, P], F32, tag="eye")
        beye_s = cpool.tile([P, P], F32, tag="beye")
        nbeye_s = cpool.tile([P, P], F32, tag="nbeye")
        w2bd_s = cpool.tile([P, P], F32, tag="w2bd")
        w3bd_s = cpool.tile([P, PO], F32, tag="w3bd")
        b1_s = cpool.tile([P, 1], F32, tag="b1")
        b2_s = cpool.tile([P, 1], F32, tag="b2")
        b3_s = cpool.tile([PO, 1], F32, tag="b3")
        nthr_s = cpool.tile([P, 1], F32, tag="nthr")
        x1f = cpool.tile([P, COLS], F32, tag="x1f")
        m3sb = cpool.tile([TP, NCH * TC * GO], F32, tag="m3sb")
        for t_s, t_d in ((eye_s, eye_d), (beye_s, beye_d), (nbeye_s, nbeye_d),
                         (w2bd_s, w2bd_d), (w3bd_s, w3bd_d),
                         (b1_s, b1_d), (b2_s, b2_d), (b3_s, b3_d)):
            nc.sync.dma_start(t_s[:], t_d[:])

        # ---------------- phase 1: x1 = W1 @ xT + b1 ----------------
        with tc.tile_pool(name="ph1", bufs=1) as xp, \
             tc.tile_pool(name="ph1ps", bufs=2, space="PSUM") as pp:
            xts, w1ts = [], []
            koff = 0
            for i, kc in enumerate(cfg.kcs):
                xt_t = xp.tile([kc, Bloc], F32, tag=f"xt{i}")
                nc.sync.dma_start(xt_t[:], xT_d[koff:koff + kc, :])
                w1_t = xp.tile([kc, F], F32, tag=f"w1{i}")
                nc.sync.dma_start(w1_t[:], w1t_d[koff:koff + kc, :])
                xts.append(xt_t)
                w1ts.append(w1_t)
                koff += kc
            for j in range(NCH):
                ps = pp.tile([P, CH], F32, tag="x1ps")
                for g in range(G):
                    for i, kc in enumerate(cfg.kcs):
                        # columns of group g in chunk j: blocks of TP at
                        # stride G*TP, offset g*TP
                        xv = xts[i][:].rearrange("p (j g u) -> p j g u",
                                                 g=G, u=TP)
                        nc.tensor.matmul(
                            ps[g * F:(g + 1) * F, :], w1ts[i][:],
                            xv[:, j * TC:(j + 1) * TC, g, :],
                            start=(i == 0), stop=(i == len(cfg.kcs) - 1))
                nc.scalar.add(x1f[:, j * CH:(j + 1) * CH], ps[:], b1_s[:])

        # ---------------- state pools ----------------
        sp = es.enter_context(tc.tile_pool(name="state", bufs=3))
        bp = es.enter_context(tc.tile_pool(name="bside", bufs=3))
        pm1 = es.enter_context(tc.tile_pool(name="pm1", bufs=2, space="PSUM"))
        pm2 = es.enter_context(tc.tile_pool(name="pm2", bufs=2, space="PSUM"))
        ph3 = es.enter_context(tc.tile_pool(name="ph3", bufs=1, space="PSUM"))
        pt12 = es.enter_context(tc.tile_pool(name="pt12", bufs=2, space="PSUM"))
        pt3 = es.enter_context(tc.tile_pool(name="pt3", bufs=1, space="PSUM"))

        m1_prev = sp.tile([P, COLS], F32, tag="m1")
        m2_prev = sp.tile([P, COLS], F32, tag="m2")
        s1_prev = sp.tile([P, COLS], F32, tag="s1")
        s2_prev = sp.tile([P, COLS], F32, tag="s2")
        for z in (m1_prev, m2_prev, s1_prev, s2_prev):
            nc.vector.memset(z[:], 0.0)
        nc.vector.memset(nthr_s[:], -THR)
        nc.vector.memset(m3sb[:], 0.0)

        for t in range(steps):
            m1_new = sp.tile([P, COLS], F32, tag="m1")
            m2_new = sp.tile([P, COLS], F32, tag="m2")
            s1_new = sp.tile([P, COLS], F32, tag="s1")
            s2_new = sp.tile([P, COLS], F32, tag="s2")
            for h in range(NCH):
                cs = slice(h * CH, (h + 1) * CH)
                # ---- L1 membrane: m1' = beta*m1 - beta*s1_prev + x1
                q1 = pm1.tile([P, CH], F32, tag="pm1")
                nc.tensor.matmul(q1[:], beye_s[:], m1_prev[:, cs],
                                 start=True, stop=False)
                nc.tensor.matmul(q1[:], nbeye_s[:], s1_prev[:, cs],
                                 start=False, stop=False)
                nc.tensor.matmul(q1[:], eye_s[:], x1f[:, cs],
                                 start=False, stop=True)
                nc.scalar.copy(m1_new[:, cs], q1[:])
                nc.vector.tensor_scalar(s1_new[:, cs], m1_new[:, cs],
                                        THR, None, AL.is_gt)
                # ---- L2 membrane: m2' = beta*m2 - beta*s2_prev + W2@s1 (+b2)
                q2 = pm2.tile([P, CH], F32, tag="pm2")
                nc.tensor.matmul(q2[:], beye_s[:], m2_prev[:, cs],
                                 start=True, stop=False)
                nc.tensor.matmul(q2[:], nbeye_s[:], s2_prev[:, cs],
                                 start=False, stop=False)
                nc.tensor.matmul(q2[:], w2bd_s[:], s1_new[:, cs],
                                 start=False, stop=True)
                nc.scalar.add(m2_new[:, cs], q2[:], b2_s[:])
                nc.vector.tensor_scalar(s2_new[:, cs], m2_new[:, cs],
                                        THR, None, AL.is_gt)
                # ---- L3 feed-forward: h3 = W3@s2 (+b3)
                q3 = ph3.tile([PO, CH], F32, tag="ph3")
                nc.tensor.matmul(q3[:], w3bd_s[:], s2_new[:, cs],
                                 start=True, stop=True)
                h3s = bp.tile([PO, CH], F32, tag="h3sb")
                nc.scalar.add(h3s[:], q3[:], b3_s[:])

                # ---- transpose m1' to batch-major; spikes + sigmoids
                tb1 = pt12.tile([TP, TC * P], F32, tag="pt12")
                for k in range(TC):
                    nc.tensor.transpose(
                        tb1[:, k * P:(k + 1) * P],
                        m1_new[:, h * CH + k * TP: h * CH + (k + 1) * TP],
                        eye_s[:])
                s1b = bp.tile([TP, TC * GF], F32, tag="s1b")
                nc.vector.tensor_scalar(s1b[:], tb1[:], THR, None, AL.is_gt)
                ss1b = bp.tile([TP, TC * GF], F32, tag="ss1b")
                nc.scalar.activation(ss1b[:], tb1[:],
                                     mybir.ActivationFunctionType.Sigmoid,
                                     bias=nthr_s[:])
                nc.sync.dma_start(
                    v_spk1[t][h],
                    s1b[:])
                nc.sync.dma_start(
                    v_ss1[t][h],
                    ss1b[:])

                # ---- transpose m2'
                tb2 = pt12.tile([TP, TC * P], F32, tag="pt12")
                for k in range(TC):
                    nc.tensor.transpose(
                        tb2[:, k * P:(k + 1) * P],
                        m2_new[:, h * CH + k * TP: h * CH + (k + 1) * TP],
                        eye_s[:])
                s2b = bp.tile([TP, TC * GF], F32, tag="s2b")
                nc.vector.tensor_scalar(s2b[:], tb2[:], THR, None, AL.is_gt)
                ss2b = bp.tile([TP, TC * GF], F32, tag="ss2b")
                nc.scalar.activation(ss2b[:], tb2[:],
                                     mybir.ActivationFunctionType.Sigmoid,
                                     bias=nthr_s[:])
                nc.sync.dma_start(
                    v_spk2[t][h],
                    s2b[:])
                nc.sync.dma_start(
                    v_ss2[t][h],
                    ss2b[:])

                # ---- L3 batch-major LIF
                t3 = pt3.tile([TP, TC * PO], F32, tag="pt3")
                for k in range(TC):
                    nc.tensor.transpose(
                        t3[:, k * PO:(k + 1) * PO],
                        h3s[:, k * TP:(k + 1) * TP],
                        eye_s[0:PO, 0:PO])
                slab = m3sb[:, h * TC * GO:(h + 1) * TC * GO]
                m3t = bp.tile([TP, TC * GO], F32, tag="m3t")
                nc.vector.scalar_tensor_tensor(m3t[:], slab, BETA, t3[:],
                                               AL.mult, AL.add)
                s3b = bp.tile([TP, TC * GO], F32, tag="s3b")
                nc.vector.tensor_scalar(s3b[:], m3t[:], THR, None, AL.is_gt)
                ss3b = bp.tile([TP, TC * GO], F32, tag="ss3b")
                nc.scalar.activation(ss3b[:], m3t[:],
                                     mybir.ActivationFunctionType.Sigmoid,
                                     bias=nthr_s[:])
                nc.vector.tensor_tensor(slab, m3t[:], s3b[:], AL.subtract)
                nc.sync.dma_start(
                    v_spko[t][h],
                    s3b[:])
                nc.sync.dma_start(
                    v_ss3[t][h],
                    ss3b[:])

            m1_prev, m2_prev = m1_new, m2_new
            s1_prev, s2_prev = s1_new, s2_new

    nc.compile()
    out_names = ["spk_out", "spk1", "spk2", "ss1", "ss2", "ss3"]
    return nc, out_names


def host_inputs(cfg: Cfg, x_shard, W1, b1, W2, b2, W3, b3):
    """Builds the per-core input map (numpy data marshalling only)."""
    G, F, O, P, PO = cfg.G, cfg.F, cfg.O, cfg.P, cfg.PO
    f32 = np.float32
    xT = np.ascontiguousarray(x_shard.T, dtype=f32)          # [Kin, Bloc]
    w1t = np.ascontiguousarray(W1.T, dtype=f32)              # [Kin, F]
    w2bd = np.zeros((P, P), f32)
    w2bd[0:F, 0:F] = W2.T
    w2bd[F:P, F:P] = W2.T
    w3bd = np.zeros((P, PO), f32)
    w3bd[0:F, 0:O] = W3.T
    w3bd[F:P, O:PO] = W3.T
    eye = np.eye(P, dtype=f32)
    return {
        "xT": xT,
        "w1t": w1t,
        "w2bd": w2bd,
        "w3bd": w3bd,
        "eye": eye,
        "beye": (f32(BETA) * eye).astype(f32),
        "nbeye": (f32(-BETA) * eye).astype(f32),
        "b1p": np.tile(np.asarray(b1, f32), G).reshape(P, 1),
        "b2p": np.tile(np.asarray(b2, f32), G).reshape(P, 1),
        "b3p": np.tile(np.asarray(b3, f32), G).reshape(PO, 1),
    }


_CACHE = {}


def kernel_with_results(x, W1, b1, W2, b2, W3, b3, **run_kwargs):
    from concourse.bass_utils import run_bass_kernel_spmd

    x = np.asarray(x, np.float32)
    B = x.shape[0]
    assert B == B_FULL and x.shape[1] == KIN_FULL
    Bloc = B // N_CORES
    cfg = Cfg(Bloc=Bloc, Kin=KIN_FULL, steps=STEPS_FULL, CH=512)

    key = "full"
    if key not in _CACHE:
        _CACHE[key] = build_nc(cfg)
    nc, out_names = _CACHE[key]

    in_maps = []
    for r in range(N_CORES):
        shard = x[r * Bloc:(r + 1) * Bloc]
        in_maps.append(host_inputs(cfg, shard, W1, b1, W2, b2, W3, b3))

    res = run_bass_kernel_spmd(nc, in_maps, core_ids=list(range(N_CORES)),
                               **run_kwargs)
    outs = []
    for name in out_names:
        outs.append(np.concatenate([np.asarray(r[name]) for r in res.results],
                                   axis=1))
    return tuple(outs), res


def kernel(x, W1, b1, W2, b2, W3, b3):
    outs, _ = kernel_with_results(x, W1, b1, W2, b2, W3, b3)
    return outs


# revision 12
# speedup vs baseline: 1.0715x; 1.0715x over previous
"""Trainium2 Bass kernel for nn_Classifier (25-step LIF SNN, 784->64->64->10).

Strategy (pure data parallel over batch, 8 cores, Bloc=2048 rows/core):

Host side (numpy, inside kernel()): shard batch, pre-transpose x -> xT,
build layout constants (W1^T, blockdiag(W2^T), blockdiag(W3^T), identity,
beta*I, -beta*I, packed biases).

Device side per core, "packed feature-major" layout for layers 1/2:
tensors are [128 partitions = 2 groups x 64 features, 1024 cols], where
column c of group g is local batch row g*1024 + c.

  phase 1:  x1 = W1 @ xT + b1 accumulated over 7 K-chunks on the PE,
            bias fused into the PSUM->SBUF copy on the scalar engine.
  step loop (t = 0..24), split into 2 column-chunks of 512:
    L1: m1' = beta*I@m1 + (-beta*I)@s1_prev + I@x1   (PE, PSUM accumulate)
        m1 SBUF copy (ACT), s1 = m1' > 1 (DVE)
    L2: m2' = beta*I@m2 + (-beta*I)@s2_prev + W2bd@s1 (PE), +b2 fused in
        the ACT copy, s2 = m2' > 1 (DVE)
    L3: h3 = W3bd@s2 (PE), +b3 in ACT copy; transposed to batch-major and
        the LIF recurrence runs on DVE (scalar_tensor_tensor etc).
    Outputs: m1'/m2' are transposed to batch-major via PE transpose-mode;
    spikes (DVE is_gt) and surrogate sigmoids (ACT) are computed from the
    transposed PSUM and streamed to DRAM per step via HWDGE DMA.

The membrane recurrence itself stays bit-faithful to the reference except
for matmul accumulation order (unavoidable) and the PSUM-accumulated
beta-decay (<=1-2 ulp/step).
"""

from contextlib import ExitStack

import numpy as np

import concourse.bass as bass
import concourse.bacc as bacc
import concourse.mybir as mybir
import concourse.tile as tile

F32 = mybir.dt.float32
BETA = 0.95
THR = 1.0

# full-size problem constants
B_FULL = 16384
N_CORES = 8
KIN_FULL = 784
F_HID = 64
O_OUT = 10
STEPS_FULL = 25


class Cfg:
    def __init__(self, Bloc, Kin, steps, CH, out_steps=None):
        self.Bloc = Bloc          # local batch rows per core
        self.Kin = Kin            # input features (784)
        self.steps = steps
        # DRAM output depth; steps beyond this wrap (timing builds only)
        self.out_steps = out_steps or steps
        self.G = 2                # packed groups on partitions
        self.F = F_HID
        self.O = O_OUT
        self.P = self.G * self.F  # 128
        self.PO = self.G * self.O  # 20
        assert Bloc % self.G == 0
        self.COLS = Bloc // self.G   # packed columns
        self.CH = CH                 # column chunk per matmul/psum tile
        assert self.COLS % CH == 0
        self.NCH = self.COLS // CH
        self.TP = 128                # transpose block width (input cols)
        assert CH % self.TP == 0
        self.TC = CH // self.TP      # transpose blocks per chunk
        # K chunking for phase 1
        self.kcs = []
        k = Kin
        while k > 0:
            c = min(128, k)
            self.kcs.append(c)
            k -= c


def build_nc(cfg: Cfg):
    """Builds the per-core SPMD Bass program. Returns (nc, in_names, out_names)."""
    G, F, O, P, PO = cfg.G, cfg.F, cfg.O, cfg.P, cfg.PO
    COLS, CH, NCH, TP, TC = cfg.COLS, cfg.CH, cfg.NCH, cfg.TP, cfg.TC
    steps, Kin, Bloc = cfg.steps, cfg.Kin, cfg.Bloc
    GF = G * F
    GO = G * O
    AL = mybir.AluOpType

    nc = bacc.Bacc("TRN2", target_bir_lowering=False, debug=False,
                   enable_asserts=False)

    # ---- DRAM parameters (per core) ----
    xT_d = nc.declare_dram_parameter("xT", [Kin, Bloc], F32, isOutput=False)
    w1t_d = nc.declare_dram_parameter("w1t", [Kin, F], F32, isOutput=False)
    w2bd_d = nc.declare_dram_parameter("w2bd", [P, P], F32, isOutput=False)
    w3bd_d = nc.declare_dram_parameter("w3bd", [P, PO], F32, isOutput=False)
    eye_d = nc.declare_dram_parameter("eye", [P, P], F32, isOutput=False)
    beye_d = nc.declare_dram_parameter("beye", [P, P], F32, isOutput=False)
    nbeye_d = nc.declare_dram_parameter("nbeye", [P, P], F32, isOutput=False)
    b1_d = nc.declare_dram_parameter("b1p", [P, 1], F32, isOutput=False)
    b2_d = nc.declare_dram_parameter("b2p", [P, 1], F32, isOutput=False)
    b3_d = nc.declare_dram_parameter("b3p", [PO, 1], F32, isOutput=False)

    osteps = cfg.out_steps
    spko_d = nc.declare_dram_parameter("spk_out", [osteps, Bloc, O], F32, isOutput=True)
    spk1_d = nc.declare_dram_parameter("spk1", [osteps, Bloc, F], F32, isOutput=True)
    spk2_d = nc.declare_dram_parameter("spk2", [osteps, Bloc, F], F32, isOutput=True)
    ss1_d = nc.declare_dram_parameter("ss1", [osteps, Bloc, F], F32, isOutput=True)
    ss2_d = nc.declare_dram_parameter("ss2", [osteps, Bloc, F], F32, isOutput=True)
    ss3_d = nc.declare_dram_parameter("ss3", [osteps, Bloc, O], F32, isOutput=True)

    # batch-major DRAM views. Packed column c = j*TP + u (j = global
    # 128-col block) of partition-group g maps to local batch row
    # r = j*(G*TP) + g*TP + u, so per-(t, h) DMA APs stay 3-dimensional
    # (the k and g strides merge: g stride TP*f, k stride G*TP*f).
    def bview(d):
        return d[:].rearrange("t (h k g c) f -> t h c k g f",
                              h=NCH, k=TC, g=G, c=TP)

    v_spk1, v_spk2, v_ss1, v_ss2 = map(bview, (spk1_d, spk2_d, ss1_d, ss2_d))
    v_spko, v_ss3 = map(bview, (spko_d, ss3_d))

    with tile.TileContext(nc) as tc, ExitStack() as es:
        cpool = es.enter_context(tc.tile_pool(name="const", bufs=1))
        eye_s = cpool.tile([P, P], F32, tag="eye")
        beye_s = cpool.tile([P, P], F32, tag="beye")
        nbeye_s = cpool.tile([P, P], F32, tag="nbeye")
        w2bd_s = cpool.tile([P, P], F32, tag="w2bd")
        w3bd_s = cpool.tile([P, PO], F32, tag="w3bd")
        b1_s = cpool.tile([P, 1], F32, tag="b1")
        b2_s = cpool.tile([P, 1], F32, tag="b2")
        b3_s = cpool.tile([PO, 1], F32, tag="b3")
        nthr_s = cpool.tile([P, 1], F32, tag="nthr")
        x1f = cpool.tile([P, COLS], F32, tag="x1f")
        m3sb = cpool.tile([TP, NCH * TC * GO], F32, tag="m3sb")
        for t_s, t_d in ((eye_s, eye_d), (beye_s, beye_d), (nbeye_s, nbeye_d),
                         (w2bd_s, w2bd_d), (w3bd_s, w3bd_d),
                         (b1_s, b1_d), (b2_s, b2_d), (b3_s, b3_d)):
            nc.sync.dma_start(t_s[:], t_d[:])

        # ---------------- phase 1: x1 = W1 @ xT + b1 ----------------
        with tc.tile_pool(name="ph1", bufs=1) as xp, \
             tc.tile_pool(name="ph1ps", bufs=2, space="PSUM") as pp:
            xts, w1ts = [], []
            koff = 0
            for i, kc in enumerate(cfg.kcs):
                xt_t = xp.tile([kc, Bloc], F32, tag=f"xt{i}")
                nc.sync.dma_start(xt_t[:], xT_d[koff:koff + kc, :])
                w1_t = xp.tile([kc, F], F32, tag=f"w1{i}")
                nc.sync.dma_start(w1_t[:], w1t_d[koff:koff + kc, :])
                xts.append(xt_t)
                w1ts.append(w1_t)
                koff += kc
            for j in range(NCH):
                ps = pp.tile([P, CH], F32, tag="x1ps")
                for g in range(G):
                    for i, kc in enumerate(cfg.kcs):
                        # columns of group g in chunk j: blocks of TP at
                        # stride G*TP, offset g*TP
                        xv = xts[i][:].rearrange("p (j g u) -> p j g u",
                                                 g=G, u=TP)
                        nc.tensor.matmul(
                            ps[g * F:(g + 1) * F, :], w1ts[i][:],
                            xv[:, j * TC:(j + 1) * TC, g, :],
                            start=(i == 0), stop=(i == len(cfg.kcs) - 1))
                nc.scalar.add(x1f[:, j * CH:(j + 1) * CH], ps[:], b1_s[:])

        # ---------------- state pools ----------------
        sp = es.enter_context(tc.tile_pool(name="state", bufs=3))
        bp = es.enter_context(tc.tile_pool(name="bside", bufs=3))
        pm1 = es.enter_context(tc.tile_pool(name="pm1", bufs=2, space="PSUM"))
        pm2 = es.enter_context(tc.tile_pool(name="pm2", bufs=2, space="PSUM"))
        ph3 = es.enter_context(tc.tile_pool(name="ph3", bufs=1, space="PSUM"))
        pt12 = es.enter_context(tc.tile_pool(name="pt12", bufs=2, space="PSUM"))
        pt3 = es.enter_context(tc.tile_pool(name="pt3", bufs=1, space="PSUM"))

        m1_prev = sp.tile([P, COLS], F32, tag="m1")
        m2_prev = sp.tile([P, COLS], F32, tag="m2")
        s1_prev = sp.tile([P, COLS], F32, tag="s1")
        s2_prev = sp.tile([P, COLS], F32, tag="s2")
        for z in (m1_prev, m2_prev, s1_prev, s2_prev):
            nc.vector.memset(z[:], 0.0)
        nc.vector.memset(nthr_s[:], -THR)
        nc.vector.memset(m3sb[:], 0.0)

        for t_ in range(steps):
            t = t_ % osteps
            m1_new = sp.tile([P, COLS], F32, tag="m1")
            m2_new = sp.tile([P, COLS], F32, tag="m2")
            s1_new = sp.tile([P, COLS], F32, tag="s1")
            s2_new = sp.tile([P, COLS], F32, tag="s2")
            for h in range(NCH):
                cs = slice(h * CH, (h + 1) * CH)
                # ---- L1 membrane: m1' = beta*m1 - beta*s1_prev + x1
                q1 = pm1.tile([P, CH], F32, tag="pm1")
                nc.tensor.matmul(q1[:], beye_s[:], m1_prev[:, cs],
                                 start=True, stop=False)
                nc.tensor.matmul(q1[:], nbeye_s[:], s1_prev[:, cs],
                                 start=False, stop=False)
                nc.tensor.matmul(q1[:], eye_s[:], x1f[:, cs],
                                 start=False, stop=True)
                nc.scalar.copy(m1_new[:, cs], q1[:])
                nc.vector.tensor_scalar(s1_new[:, cs], m1_new[:, cs],
                                        THR, None, AL.is_gt)
                # ---- L2 membrane: m2' = beta*m2 - beta*s2_prev + W2@s1 (+b2)
                q2 = pm2.tile([P, CH], F32, tag="pm2")
                nc.tensor.matmul(q2[:], beye_s[:], m2_prev[:, cs],
                                 start=True, stop=False)
                nc.tensor.matmul(q2[:], nbeye_s[:], s2_prev[:, cs],
                                 start=False, stop=False)
                nc.tensor.matmul(q2[:], w2bd_s[:], s1_new[:, cs],
                                 start=False, stop=True)
                nc.scalar.add(m2_new[:, cs], q2[:], b2_s[:])
                nc.vector.tensor_scalar(s2_new[:, cs], m2_new[:, cs],
                                        THR, None, AL.is_gt)
                # ---- L3 feed-forward: h3 = W3@s2 (+b3)
                q3 = ph3.tile([PO, CH], F32, tag="ph3")
                nc.tensor.matmul(q3[:], w3bd_s[:], s2_new[:, cs],
                                 start=True, stop=True)
                h3s = bp.tile([PO, CH], F32, tag="h3sb")
                nc.scalar.add(h3s[:], q3[:], b3_s[:])

                # ---- transpose m1' to batch-major; spikes + sigmoids
                tb1 = pt12.tile([TP, TC * P], F32, tag="pt12")
                for k in range(TC):
                    nc.tensor.transpose(
                        tb1[:, k * P:(k + 1) * P],
                        m1_new[:, h * CH + k * TP: h * CH + (k + 1) * TP],
                        eye_s[:])
                s1b = bp.tile([TP, TC * GF], F32, tag="s1b")
                nc.vector.tensor_scalar(s1b[:], tb1[:], THR, None, AL.is_gt)
                ss1b = bp.tile([TP, TC * GF], F32, tag="ss1b")
                nc.scalar.activation(ss1b[:], tb1[:],
                                     mybir.ActivationFunctionType.Sigmoid,
                                     bias=nthr_s[:])
                nc.sync.dma_start(
                    v_spk1[t][h],
                    s1b[:])
                nc.sync.dma_start(
                    v_ss1[t][h],
                    ss1b[:])

                # ---- transpose m2'
                tb2 = pt12.tile([TP, TC * P], F32, tag="pt12")
                for k in range(TC):
                    nc.tensor.transpose(
                        tb2[:, k * P:(k + 1) * P],
                        m2_new[:, h * CH + k * TP: h * CH + (k + 1) * TP],
                        eye_s[:])
                s2b = bp.tile([TP, TC * GF], F32, tag="s2b")
                nc.vector.tensor_scalar(s2b[:], tb2[:], THR, None, AL.is_gt)
                ss2b = bp.tile([TP, TC * GF], F32, tag="ss2b")
                nc.scalar.activation(ss2b[:], tb2[:],
                                     mybir.ActivationFunctionType.Sigmoid,
                                     bias=nthr_s[:])
                nc.sync.dma_start(
                    v_spk2[t][h],
                    s2b[:])
                nc.sync.dma_start(
                    v_ss2[t][h],
                    ss2b[:])

                # ---- L3 batch-major LIF
                t3 = pt3.tile([TP, TC * PO], F32, tag="pt3")
                for k in range(TC):
                    nc.tensor.transpose(
                        t3[:, k * PO:(k + 1) * PO],
                        h3s[:, k * TP:(k + 1) * TP],
                        eye_s[0:PO, 0:PO])
                slab = m3sb[:, h * TC * GO:(h + 1) * TC * GO]
                m3t = bp.tile([TP, TC * GO], F32, tag="m3t")
                nc.vector.scalar_tensor_tensor(m3t[:], slab, BETA, t3[:],
                                               AL.mult, AL.add)
                s3b = bp.tile([TP, TC * GO], F32, tag="s3b")
                nc.vector.tensor_scalar(s3b[:], m3t[:], THR, None, AL.is_gt)
                ss3b = bp.tile([TP, TC * GO], F32, tag="ss3b")
                nc.scalar.activation(ss3b[:], m3t[:],
                                     mybir.ActivationFunctionType.Sigmoid,
                                     bias=nthr_s[:])
                nc.vector.tensor_tensor(slab, m3t[:], s3b[:], AL.subtract)
                nc.sync.dma_start(
                    v_spko[t][h],
                    s3b[:])
                nc.sync.dma_start(
                    v_ss3[t][h],
                    ss3b[:])

            m1_prev, m2_prev = m1_new, m2_new
            s1_prev, s2_prev = s1_new, s2_new

    nc.compile()
    out_names = ["spk_out", "spk1", "spk2", "ss1", "ss2", "ss3"]
    return nc, out_names


def host_inputs(cfg: Cfg, x_shard, W1, b1, W2, b2, W3, b3):
    """Builds the per-core input map (numpy data marshalling only)."""
    G, F, O, P, PO = cfg.G, cfg.F, cfg.O, cfg.P, cfg.PO
    f32 = np.float32
    xT = np.ascontiguousarray(x_shard.T, dtype=f32)          # [Kin, Bloc]
    w1t = np.ascontiguousarray(W1.T, dtype=f32)              # [Kin, F]
    w2bd = np.zeros((P, P), f32)
    w2bd[0:F, 0:F] = W2.T
    w2bd[F:P, F:P] = W2.T
    w3bd = np.zeros((P, PO), f32)
    w3bd[0:F, 0:O] = W3.T
    w3bd[F:P, O:PO] = W3.T
    eye = np.eye(P, dtype=f32)
    return {
        "xT": xT,
        "w1t": w1t,
        "w2bd": w2bd,
        "w3bd": w3bd,
        "eye": eye,
        "beye": (f32(BETA) * eye).astype(f32),
        "nbeye": (f32(-BETA) * eye).astype(f32),
        "b1p": np.tile(np.asarray(b1, f32), G).reshape(P, 1),
        "b2p": np.tile(np.asarray(b2, f32), G).reshape(P, 1),
        "b3p": np.tile(np.asarray(b3, f32), G).reshape(PO, 1),
    }


_CACHE = {}


def kernel_with_results(x, W1, b1, W2, b2, W3, b3, **run_kwargs):
    from concourse.bass_utils import run_bass_kernel_spmd

    x = np.asarray(x, np.float32)
    B = x.shape[0]
    assert B == B_FULL and x.shape[1] == KIN_FULL
    Bloc = B // N_CORES
    cfg = Cfg(Bloc=Bloc, Kin=KIN_FULL, steps=STEPS_FULL, CH=512)

    key = "full"
    if key not in _CACHE:
        _CACHE[key] = build_nc(cfg)
    nc, out_names = _CACHE[key]

    in_maps = []
    for r in range(N_CORES):
        shard = x[r * Bloc:(r + 1) * Bloc]
        in_maps.append(host_inputs(cfg, shard, W1, b1, W2, b2, W3, b3))

    res = run_bass_kernel_spmd(nc, in_maps, core_ids=list(range(N_CORES)),
                               **run_kwargs)
    outs = []
    for name in out_names:
        outs.append(np.concatenate([np.asarray(r[name]) for r in res.results],
                                   axis=1))
    return tuple(outs), res


def kernel(x, W1, b1, W2, b2, W3, b3):
    outs, _ = kernel_with_results(x, W1, b1, W2, b2, W3, b3)
    return outs
